# revision 1
# baseline (speedup 1.0000x reference)
"""Cross-attention TRN2 Bass kernel (nn_CrossAttention).

Full-input contract: kernel(**inputs) takes the unsharded numpy inputs and
returns the full output. Internally shards across 8 NeuronCores:
  core c -> batch b = c // 4, heads h0 = (c % 4) * 4 .. h0+3  (B=2, H=16)

Per-core device program (all matmuls in float32r = fp32 storage, 11-bit
mantissa multiplies, full PE rate at N>=256):
  phase 1: qT = Wq_h @ y.T + bq  -> [256, 2048]   (heads on partitions)
           kT = Wk_h @ x.T       -> [256, 2048]   (k bias dropped: it shifts
                                                   each softmax row by a
                                                   constant -> cancels)
           v  = x @ Wv_h.T       -> [2048, 256]   (natural layout, v bias is
                                                   added on host: sum w_i=1)
  phase 2: per head: attT[d,y] = kT^T-slice matmuls; exp on ACT;
           outT[p,y] (+ denominator row via ones column in the stationary
           v tile) accumulated over d tiles.
Host: normalize by the denominator row, add v bias, reassemble the
reference's raw (B, H*Dy*pd) reshape.
"""

import numpy as np

import concourse.bacc as bacc
import concourse.tile as tile
from concourse import mybir
from concourse.bass_utils import run_bass_kernel_spmd

DIM = 1024
H = 16
B = 2
SEQ = 2048  # both SEQ_X and SEQ_Y
PD = 64  # head dim
HPC = 4  # heads per core
PCOLS = HPC * PD  # 256 projection columns per core
N_CORES = 8

F32 = mybir.dt.float32
F32R = mybir.dt.float32r

_NC_CACHE = None


def _round_f32r(a: np.ndarray) -> np.ndarray:
    """Round fp32 -> float32r bit pattern (RNE, drop low 12 mantissa bits).

    Matches the hardware rounding verified on-device (DVE fp32->f32r copy).
    """
    b = np.ascontiguousarray(a, dtype=np.float32).view(np.uint32).astype(np.uint64)
    half = np.uint64(1 << 11)
    lsb_mask = np.uint64((1 << 12) - 1)
    rounded = (b + half - np.uint64(1) + ((b >> np.uint64(12)) & np.uint64(1))) & ~lsb_mask
    return rounded.astype(np.uint32).view(np.float32).reshape(a.shape)


def _build_nc(repeat=1, loop_n=0, variant="full"):
    nc = bacc.Bacc(trn_type="TRN2", name="cross_attention")

    yt = nc.dram_tensor("yt", [DIM, SEQ], F32R, kind="ExternalInput")
    xt = nc.dram_tensor("xt", [DIM, SEQ], F32R, kind="ExternalInput")
    wqt = nc.dram_tensor("wqt", [DIM, PCOLS], F32R, kind="ExternalInput")
    wkt = nc.dram_tensor("wkt", [DIM, PCOLS], F32R, kind="ExternalInput")
    wvt = nc.dram_tensor("wvt", [DIM, PCOLS], F32R, kind="ExternalInput")
    bq = nc.dram_tensor("bq", [PCOLS, 1], F32, kind="ExternalInput")
    o = nc.dram_tensor("o", [HPC, PD + 1, SEQ], F32, kind="ExternalOutput")

    NC = DIM // 128  # 8 c-tiles
    ND = SEQ // 128  # 16 d-tiles
    NY = SEQ // 512  # 4 y-chunks of 512

    with tile.TileContext(nc) as tc:
        with (
            tc.tile_pool(name="persist", bufs=1) as pp,
            tc.tile_pool(name="ytp", bufs=2) as ytp,
            tc.tile_pool(name="attexp", bufs=5) as aep,
            tc.tile_pool(name="outp", bufs=1) as outp,
            tc.tile_pool(name="ps_att", bufs=3, space="PSUM") as ps_att,
            tc.tile_pool(name="ps_o", bufs=1, space="PSUM") as ps_o,
        ):
            if loop_n:
                import contextlib
                loop_cm = tc.For_i(0, loop_n, 1)
            else:
                loop_cm = None
            with (loop_cm if loop_cm is not None else __import__("contextlib").nullcontext()):
              for rep in range(repeat):
                # ---- resident loads ----
                # Wire-order priority: wq+wk (gate the first projections),
                # then xt (gates k proj + v proj), then wv+bq. yt halves are
                # DMA'd inside the q-projection passes.
                wq_sb = []
                wk_sb = []
                wv_sb = []
                for c in range(NC):
                    t = pp.tile([128, PCOLS], F32R, tag=f"wq{c}", name=f"wq{c}_r{rep}")
                    nc.gpsimd.dma_start(out=t, in_=wqt.ap()[c * 128:(c + 1) * 128, :])
                    wq_sb.append(t)
                for c in range(NC):
                    t = pp.tile([128, PCOLS], F32R, tag=f"wk{c}", name=f"wk{c}_r{rep}")
                    nc.gpsimd.dma_start(out=t, in_=wkt.ap()[c * 128:(c + 1) * 128, :])
                    wk_sb.append(t)
                # xt as two half-tiles per c so the first attention chunks
                # only wait on the first 1024 seq columns of x; half 1 is
                # emitted later in the wire order (see emission sequence)
                xt_half = [[None] * NC for _ in range(2)]

                def emit_xt(half):
                    for c in range(NC):
                        t = pp.tile([128, SEQ // 2], F32R, tag=f"xt{half}_{c}",
                                    name=f"xt{half}_{c}_r{rep}")
                        nc.scalar.dma_start(
                            out=t,
                            in_=xt.ap()[c * 128:(c + 1) * 128,
                                        half * (SEQ // 2):(half + 1) * (SEQ // 2)])
                        xt_half[half][c] = t

                def xt_slice(c, lo, hi):
                    half = lo // (SEQ // 2)
                    assert hi <= (half + 1) * (SEQ // 2)
                    base = half * (SEQ // 2)
                    return xt_half[half][c][:, lo - base:hi - base]

                emit_xt(0)
                bq_sb = []
                for m in range(2):
                    t = pp.tile([128, 1], F32, tag=f"bq{m}", name=f"bq{m}_r{rep}")
                    nc.gpsimd.dma_start(out=t, in_=bq.ap()[m * 128:(m + 1) * 128, :])
                    bq_sb.append(t)

                def emit_wv_bq():
                    for c in range(NC):
                        t = pp.tile([128, PCOLS], F32R, tag=f"wv{c}", name=f"wv{c}_r{rep}")
                        nc.gpsimd.dma_start(out=t, in_=wvt.ap()[c * 128:(c + 1) * 128, :])
                        wv_sb.append(t)

                qT_sb = [pp.tile([128, SEQ], F32R, tag=f"qT{m}", name=f"qT{m}_r{rep}") for m in range(2)]
                # kT per head, zero-padded to K=128: the other head's 64 rows
                # are 0 so QK^T can contract the full 128 partitions (K=64
                # matmuls measure ~60% slower per-instruction on HW)
                kT_pad = [pp.tile([128, SEQ], F32R, tag=f"kp{h}", name=f"kp{h}_r{rep}") for h in range(HPC)]
                v_sb = [pp.tile([128, HPC, PD + 1], F32R, tag=f"v{d}", name=f"v{d}_r{rep}") for d in range(ND)]
                ones_sb = pp.tile([128, HPC], F32, tag="ones", name=f"ones_r{rep}")
                nc.vector.memset(ones_sb, 1.0)
                for d in range(ND):
                    nc.vector.tensor_copy(v_sb[d][:, :, PD:PD + 1], ones_sb)
                zeros_sb = ytp.tile([128, SEQ // 2], F32, tag="zeros", name=f"zeros_r{rep}", bufs=1)
                nc.vector.memset(zeros_sb, 0.0)
                for h in range(HPC):
                    ooff = 64 if h % 2 == 0 else 0
                    for yh2 in range(2):
                        nc.vector.tensor_copy(
                            kT_pad[h][ooff:ooff + 64,
                                      yh2 * (SEQ // 2):(yh2 + 1) * (SEQ // 2)],
                            zeros_sb[0:64, :])

                HY = SEQ // 2  # 1024

                # Projection psum tiles share the "pa" slots of ps_att (PSUM has
                # only 8 banks: pa 2x2 + po 2x2 fills it).
                def proj_pass(kind, m, yh):
                    """One [128, 1024] projection pass: q or k, head pair m, y half."""
                    ps = ps_att.tile([128, HY], F32, tag="pa", name=f"p{kind}{m}_{yh}_r{rep}")
                    for c in range(NC):
                        if kind == "q":
                            if variant == "nodma":
                                src_t = xt_slice(c, yh * HY, (yh + 1) * HY)
                            else:
                                src_t = ytp.tile([128, HY], F32R, tag="yt", name=f"yt{m}_{yh}_{c}_r{rep}")
                                nc.sync.dma_start(
                                    out=src_t,
                                    in_=yt.ap()[c * 128:(c + 1) * 128, yh * HY:(yh + 1) * HY])
                            w = wq_sb[c]
                        else:
                            src_t = xt_slice(c, yh * HY, (yh + 1) * HY)
                            w = wk_sb[c]
                        for n in range(2):
                            nc.tensor.matmul(
                                ps[:, n * 512:(n + 1) * 512],
                                w[:, m * 128:(m + 1) * 128],
                                src_t[:, n * 512:(n + 1) * 512],
                                start=(c == 0),
                                stop=(c == NC - 1),
                            )
                    if kind == "q":
                        dst = qT_sb[m][:, yh * HY:(yh + 1) * HY]
                        nc.vector.tensor_scalar_add(dst, ps, bq_sb[m])
                    else:
                        for j in range(2):
                            h2 = 2 * m + j
                            nc.vector.tensor_copy(
                                kT_pad[h2][j * 64:(j + 1) * 64, yh * HY:(yh + 1) * HY],
                                ps[j * 64:(j + 1) * 64, :])

                def proj_v_single(d):
                    pvt = ps_att.tile([128, PCOLS], F32, tag="pa", name=f"pvs{d}_r{rep}")
                    for c in range(NC):
                        nc.tensor.matmul(
                            pvt,
                            xt_slice(c, d * 128, (d + 1) * 128),
                            wv_sb[c],
                            start=(c == 0),
                            stop=(c == NC - 1),
                        )
                    nc.vector.tensor_copy(
                        v_sb[d][:, :, 0:PD],
                        pvt.rearrange("p (h e) -> p h e", h=HPC),
                    )

                def proj_v_pair(dpair):
                    """v projection for d-tiles (2*dpair, 2*dpair+1); borrows one
                    pa slot per d-tile for ~8 matmuls."""
                    for j in range(2):
                        d = 2 * dpair + j
                        pvt = ps_att.tile([128, PCOLS], F32, tag="pa", name=f"pv{d}_r{rep}")
                        for c in range(NC):
                            nc.tensor.matmul(
                                pvt,
                                xt_slice(c, d * 128, (d + 1) * 128),
                                wv_sb[c],
                                start=(c == 0),
                                stop=(c == NC - 1),
                            )
                        nc.vector.tensor_copy(
                            v_sb[d][:, :, 0:PD],
                            pvt.rearrange("p (h e) -> p h e", h=HPC),
                        )

                def attention_pass(h, yh, interleave=None):
                    """One (head, y-half): QK^T -> exp -> A@V (+ denom row).

                    po is a single [PD+1, 1024] tile (2 banks); pa chunks are
                    triple-buffered so interleaved projection work that
                    borrows a pa slot does not stall the exp pipeline.
                    """
                    m, off = h // 2, (h % 2) * 64
                    po = ps_o.tile([PD + 1, HY], F32, tag="po", name=f"po{h}_{yh}_r{rep}")
                    for d in range(ND):
                        if interleave and d in interleave:
                            interleave[d]()
                        pa = ps_att.tile([128, HY], F32, tag="pa", name=f"pa{h}_{d}_{yh}_r{rep}")
                        for n in range(2):
                            nc.tensor.matmul(
                                pa[:, n * 512:(n + 1) * 512],
                                kT_pad[h][:, d * 128:(d + 1) * 128],
                                qT_sb[m][:, yh * HY + n * 512:yh * HY + (n + 1) * 512],
                                start=True,
                                stop=True,
                            )
                        ae = aep.tile([128, HY], F32R, tag="ae", name=f"ae{h}_{d}_{yh}_r{rep}")
                        nc.scalar.activation(
                            out=ae,
                            in_=pa,
                            func=mybir.ActivationFunctionType.Exp,
                            scale=1.0,
                        )
                        for n in range(2):
                            nc.tensor.matmul(
                                po[:, n * 512:(n + 1) * 512],
                                v_sb[d][:, h, :],
                                ae[:, n * 512:(n + 1) * 512],
                                start=(d == 0),
                                stop=(d == ND - 1),
                            )
                    osb = outp.tile([PD + 1, HY], F32, tag="osb", name=f"osb{h}_{yh}_r{rep}")
                    nc.vector.tensor_copy(osb, po)
                    nc.sync.dma_start(
                        out=o.ap()[h, :, yh * HY:(yh + 1) * HY], in_=osb)

                # ---- emission order drives scheduling priority ----
                proj_pass("q", 0, 0)
                emit_wv_bq()
                emit_xt(1)
                proj_pass("k", 0, 0)
                il0 = {2 * i: (lambda i=i: proj_v_pair(i)) for i in range(8)}
                il0[5] = lambda: proj_pass("k", 0, 1)
                il0[11] = lambda: proj_pass("q", 0, 1)
                attention_pass(0, 0, interleave=il0)
                attention_pass(0, 1)
                attention_pass(1, 0, interleave={
                    2: lambda: proj_pass("q", 1, 0),
                    10: lambda: proj_pass("q", 1, 1),
                })
                attention_pass(1, 1, interleave={
                    2: lambda: proj_pass("k", 1, 0),
                    10: lambda: proj_pass("k", 1, 1),
                })
                attention_pass(2, 0)
                attention_pass(2, 1)
                attention_pass(3, 0)
                attention_pass(3, 1)

    nc.compile()
    return nc


def _get_nc():
    global _NC_CACHE
    if _NC_CACHE is None:
        _NC_CACHE = _build_nc()
    return _NC_CACHE


_NC_REPEAT_CACHE = {}


def _get_nc_repeat(repeat):
    if repeat not in _NC_REPEAT_CACHE:
        _NC_REPEAT_CACHE[repeat] = _build_nc(repeat)
    return _NC_REPEAT_CACHE[repeat]


_NC_LOOP_CACHE = {}


def _get_nc_loop(loop_n):
    if loop_n not in _NC_LOOP_CACHE:
        _NC_LOOP_CACHE[loop_n] = _build_nc(1, loop_n=loop_n)
    return _NC_LOOP_CACHE[loop_n]


def kernel(x, y, Wq, bq, Wkv, bkv, _collect_results=None):
    x = np.asarray(x, dtype=np.float32)
    y = np.asarray(y, dtype=np.float32)
    Wq = np.asarray(Wq, dtype=np.float32)
    bq = np.asarray(bq, dtype=np.float32)
    Wkv = np.asarray(Wkv, dtype=np.float32)
    bkv = np.asarray(bkv, dtype=np.float32)

    nc = _get_nc()

    in_maps = []
    for core in range(N_CORES):
        b = core // 4
        h0 = (core % 4) * HPC
        cs = slice(h0 * PD, h0 * PD + PCOLS)
        vs = slice(DIM + h0 * PD, DIM + h0 * PD + PCOLS)
        in_maps.append({
            "yt": _round_f32r(y[b].T),
            "xt": _round_f32r(x[b].T),
            "wqt": _round_f32r(Wq[cs, :].T),
            "wkt": _round_f32r(Wkv[cs, :].T),
            "wvt": _round_f32r(Wkv[vs, :].T),
            "bq": np.ascontiguousarray(bq[cs].reshape(PCOLS, 1)),
        })

    res = run_bass_kernel_spmd(nc, in_maps, list(range(N_CORES)))
    if _collect_results is not None:
        _collect_results.append(res)

    O = np.empty((B, H, SEQ, PD), np.float32)
    for core in range(N_CORES):
        b = core // 4
        h0 = (core % 4) * HPC
        oc = res.results[core]["o"]  # [HPC, PD+1, SEQ]
        num = oc[:, :PD, :].astype(np.float64)
        den = oc[:, PD, :].astype(np.float64)
        for i in range(HPC):
            h = h0 + i
            bv = bkv[DIM + h * PD:DIM + (h + 1) * PD]
            O[b, h] = (num[i] / den[i][None, :]).T + bv[None, :]
    return O.reshape(B, SEQ, DIM)

